# revision 15
# baseline (speedup 1.0000x reference)
"""DFSMN (order-9 IIR + 2-tap lookahead FIR along frames) on 8 Trainium2 cores.

Math: the reference computes, per (b, h, d) sequence along frames t:
    p[t] = base[t] + sum_{k=1..9} c_k[d] * p[t-k]
    base[t] = (1 + l0[d]) v[t] + r1[d] v[t+1] + r2[d] v[t+2]
This is a per-channel LTI filter, so p = w_d * v (convolution with the
filter's impulse response, which decays below bf16 resolution past lag
~120). Each 128-frame output block therefore depends only on the previous
256 input frames, which turns the whole problem into, per channel d:

    out_block(b) = W1_d^T @ x[window b] + W2_d^T @ x[window b+1]

with W1/W2 128x128 Toeplitz matrices built on the host from the impulse
response, and windows = consecutive 128-frame chunks of the shifted input.

Precision: the harness gate is rel_err < 2e-2, so everything runs in plain
bf16 (x, W, y) with fp32 PSUM accumulation -> rel err ~2e-3 and half the
HBM traffic of an fp32/hi-lo scheme.  The whole kernel is HBM-bound:
per-core traffic is x 8.4MB + w 4.2MB + y 8.4MB = 21MB -> ~59us floor.

Window 0 of the padded input is 126 zeros + v[0:2]; instead of loading it,
its rank-2 contribution to output block 0 (together with the "base does
not exist for t<0" boundary correction) is applied on the host after
gathering.  On-chip x therefore holds exactly windows 1..8 = frames
2..1025 (1022 real + 2 zero), i.e. per (channel, window, bh):

    ps[:, 0:64]   = W2 @ win0                       (block 0, host-corrected)
    ps[:, 64:512] = W2 @ win(1..7) + W1 @ win(0..6) (blocks 1..7)

All DRAM layouts are the exact SBUF layouts (host transposes are free):
every DMA is a plain 2D column slice with 8KB contiguous runs/partition.

Sharding: channels d (512) split across 8 cores (64 each); all 64 (b,h)
sequences ride the matmul free dimension. Zero cross-device communication.

Per-core tensors:
    x  [128, 64*512] bf16   col = d*512 + win*64 + bh
    w  [128, 64*256] bf16   col = d*256 + c; c<128: W1 lhsT, c>=128: W2 lhsT
    y  [128, 64*512] bf16   col = d*512 + blk*64 + bh
"""

import numpy as np

import concourse.bass as bass
import concourse.bacc as bacc
import concourse.mybir as mybir
from concourse import tile
from concourse import bass_utils

B, H, T, D = 16, 4, 1024, 512
N_CORES = 8
DC = D // N_CORES          # 64 channels per core
BH = B * H                 # 64 sequences (matmul free dim)
NBLK = T // 128            # 8 output blocks
NWIN = 8                   # windows kept on-chip (old windows 1..8)
F32 = mybir.dt.float32
BF16 = mybir.dt.bfloat16
FREE = NBLK * BH           # 512, matmul free dim
WCOL = 192                 # w cols per channel: W2 [128x128] + W1 [64x64] block

_NC_CACHE: dict = {}


def _build_nc(dc: int = DC):
    nc = bacc.Bacc("TRN2", target_bir_lowering=False, debug=False)
    x = nc.dram_tensor("x", [128, dc * FREE], BF16, kind="ExternalInput")
    w = nc.dram_tensor("w", [128, dc * WCOL], BF16, kind="ExternalInput")
    y = nc.dram_tensor("y", [128, dc * FREE], BF16, kind="ExternalOutput")
    xap, wap, yap = x.ap(), w.ap(), y.ap()
    XG, WG = 8, 8                      # channels per x-load / w-load

    with tile.TileContext(nc) as tc:
        with tc.tile_pool(name="xp", bufs=5) as xp, \
             tc.tile_pool(name="wp", bufs=4) as wp, \
             tc.tile_pool(name="op", bufs=1) as op, \
             tc.tile_pool(name="pp", bufs=7, space="PSUM") as pp, \
             tc.tile_pool(name="pwp", bufs=1, space="PSUM") as pwp:
            # all of y stays in SBUF (64KB/partition) and is written to HBM in
            # one burst at the end: HBM read/write turnaround makes the mixed
            # stream ~25% slower than reads-only + writes-only phases.
            yt = op.tile([128, dc * FREE], BF16, name="yt")
            # PE warm-up: the HAM clock gate keeps the PE at 1.2 GHz until it
            # sees ~3.4us of sustained activity; burn the dead DMA-head time
            # on dummy matmuls so the real stream runs at 2.4 GHz.
            dummy = op.tile([128, FREE], BF16, name="dummy")
            nc.gpsimd.memset(dummy, 0.0)
            pw = pwp.tile([128, FREE], F32, name="pw")
            for _ in range(14):
                nc.tensor.matmul(pw, lhsT=dummy[:, 0:128], rhs=dummy,
                                 start=True, stop=True)
            xt = wt = None
            for d in range(dc):
                if d % XG == 0:
                    g = d // XG
                    xt = xp.tile([128, XG * FREE], BF16, name="xt")
                    src = xap.copy()
                    src.ap = src.ap[:0] + [[dc * FREE, 128], [1, XG * FREE]]
                    src.offset = g * XG * FREE
                    nc.sync.dma_start(out=xt, in_=src)
                if d % WG == 0:
                    g = d // WG
                    wt = wp.tile([128, WG * WCOL], BF16, name="wt")
                    wsrc = wap.copy()
                    wsrc.ap = wsrc.ap[:0] + [[dc * WCOL, 128], [1, WG * WCOL]]
                    wsrc.offset = g * WG * WCOL
                    nc.scalar.dma_start(out=wt, in_=wsrc)
                xv = xt[:, (d % XG) * FREE:(d % XG + 1) * FREE]
                wo = (d % WG) * WCOL
                w2 = wt[:, wo:wo + 128]
                w1 = wt[64:128, wo + 128:wo + 192]     # [64, 64] lhsT
                ps = pp.tile([128, FREE], F32, name="ps")
                # ps[:, 0:64] (block 0) only gets the W2 term; its group is
                # never stop'ed -> skip the sim's accumulation-group check.
                nc.tensor.matmul(ps, lhsT=w2, rhs=xv,
                                 start=True, stop=False, skip_group_check=True)
                # W1 truncated to rows [64:128] x cols [0:64] (impulse response
                # below bf16 noise past lag ~63); contraction operands both sit
                # at partitions 64:128, output covers frames 0:64 of blocks 1-7.
                nc.tensor.matmul(ps[0:BH, BH:FREE], lhsT=w1,
                                 rhs=xv[64:128, 0:FREE - BH],
                                 start=False, stop=True, skip_group_check=True)
                # alternate PSUM evacuation between DVE and ACT
                ysl = yt[:, d * FREE:(d + 1) * FREE]
                if d % 2 == 0:
                    nc.vector.tensor_copy(ysl, ps)
                else:
                    nc.scalar.copy(ysl, ps)
            # final write burst: two contiguous halves on the two HWDGE rings.
            # sync's half includes the last channel so its sem gate fires at
            # end-of-compute; scalar's dispatch sits behind its last ACT cast
            # in program order, so neither store mixes with the read stream.
            half = dc // 2 * FREE
            for k, eng in ((0, nc.scalar), (1, nc.sync)):
                dst = yap.copy()
                dst.ap = dst.ap[:0] + [[dc * FREE, 128], [1, half]]
                dst.offset = k * half
                eng.dma_start(out=dst, in_=yt[:, k * half:(k + 1) * half])
    nc.compile()
    return nc


def _get_nc(dc: int = DC):
    if dc not in _NC_CACHE:
        _NC_CACHE[dc] = _build_nc(dc)
    return _NC_CACHE[dc]


def _build_filters(l_filter: np.ndarray, r_filter: np.ndarray):
    """Returns wmat [128, 256, D] float64 (k, i; W1 = [:, :128], W2 = [:, 128:])
    and the rank-2 block-0 boundary correction corr [2, 128, D] float64."""
    c = l_filter[1:].astype(np.float64)            # (9, D) IIR coeffs
    d = c.shape[1]
    a = np.zeros((258, d))
    a[0] = 1.0
    for n in range(1, 258):
        for k in range(1, min(9, n) + 1):
            a[n] += c[k - 1] * a[n - k]
    q0 = 1.0 + l_filter[0].astype(np.float64)
    q1 = r_filter[0].astype(np.float64)
    q2 = r_filter[1].astype(np.float64)

    # wseq[lag + 129] = combined FIR tap at lag, lag in [-129, 253] (0 < -2)
    wseq = np.zeros((383, d))
    for lag in range(-2, 254):
        t = q2 * a[lag + 2]
        if lag + 1 >= 0:
            t = t + q1 * a[lag + 1]
        if lag >= 0:
            t = t + q0 * a[lag]
        wseq[lag + 129] = t

    kk = np.arange(128)[:, None]
    ii = np.arange(128)[None, :]
    w1 = wseq[ii - kk + 255]                       # (128, 128, D)
    w2 = wseq[ii - kk + 127]
    wmat = np.concatenate([w1, w2], axis=1)        # (128, 256, D)

    i1 = np.arange(128)
    corr = np.stack([-(q1[None, :] * a[i1 + 1] + q2[None, :] * a[i1 + 2]),
                     -(q2[None, :] * a[i1 + 1])], axis=0)   # (2, 128, D)
    return wmat, corr


def _make_in_maps(v, l_filter, r_filter, n_cores=N_CORES, dc=DC):
    import ml_dtypes
    bf16 = ml_dtypes.bfloat16
    wmat, _ = _build_filters(l_filter, r_filter)
    vr = np.asarray(v, dtype=np.float32).reshape(BH, T, D)
    wb = wmat.astype(np.float32).astype(bf16)      # (128, 256, D)
    # per-channel w block: [:, 0:128] = W2 lhsT (full), [64:128, 128:192] =
    # W1 lhsT truncated to rows 64:128 x cols 0:64 (rest below bf16 noise)
    warr = np.zeros((128, D, WCOL), bf16)
    warr[:, :, 0:128] = wb[:, 128:256, :].transpose(0, 2, 1)
    warr[64:128, :, 128:192] = wb[64:128, 0:64, :].transpose(0, 2, 1)

    in_maps = []
    for cid in range(n_cores):
        sl = slice(cid * dc, (cid + 1) * dc)
        vc = vr[:, :, sl].astype(bf16)             # (BH, T, dc)
        xarr = np.zeros((128, dc, NWIN, BH), bf16)
        for win in range(NWIN):
            t0 = 128 * win + 2
            n = min(128, T - t0)
            # (BH, n, dc) -> (n, dc, BH)
            xarr[:n, :, win, :] = vc[:, t0:t0 + n, :].transpose(1, 2, 0)
        in_maps.append({
            "x": np.ascontiguousarray(xarr).reshape(128, dc * FREE),
            "w": np.ascontiguousarray(warr[:, sl, :]).reshape(128, dc * WCOL),
        })
    return in_maps


def kernel(v: np.ndarray, l_filter: np.ndarray, r_filter: np.ndarray,
           **_unused) -> np.ndarray:
    nc = _get_nc(DC)
    in_maps = _make_in_maps(v, l_filter, r_filter)
    res = bass_utils.run_bass_kernel_spmd(nc, in_maps,
                                          core_ids=list(range(N_CORES)))
    vr = np.asarray(v, dtype=np.float32).reshape(BH, T, D)
    out = np.empty((BH, T, D), np.float32)
    for cid in range(N_CORES):
        yc = np.asarray(res.results[cid]["y"]).reshape(128, DC, NBLK, BH)
        # (i, d, b, j) -> (j, b, i, d) -> (BH, T, dc)
        out[:, :, cid * DC:(cid + 1) * DC] = (
            yc.astype(np.float32).transpose(3, 2, 0, 1).reshape(BH, T, DC))

    # Block-0 boundary correction: dropped window 0 (rank-2 in v[0:2]) plus
    # the "base does not exist for t<0" fix, both exact in f64 on the host.
    wmat, corr = _build_filters(l_filter, r_filter)
    cmat = wmat[126:128, 0:128, :] + corr          # (2, 128, D)
    out[:, 0:128, :] += np.einsum(
        "mid,jmd->jid", cmat, vr[:, 0:2, :].astype(np.float64)
    ).astype(np.float32)
    return out.reshape(B, H, T, D)


# revision 16
# speedup vs baseline: 1.1304x; 1.1304x over previous
"""DFSMN (order-9 IIR + 2-tap lookahead FIR along frames) on 8 Trainium2 cores.

Math: the reference computes, per (b, h, d) sequence along frames t:
    p[t] = base[t] + sum_{k=1..9} c_k[d] * p[t-k]
    base[t] = (1 + l0[d]) v[t] + r1[d] v[t+1] + r2[d] v[t+2]
This is a per-channel LTI filter, so p = w_d * v (convolution with the
filter's impulse response, which decays below bf16 resolution past lag
~120). Each 128-frame output block therefore depends only on the previous
256 input frames, which turns the whole problem into, per channel d:

    out_block(b) = W1_d^T @ x[window b] + W2_d^T @ x[window b+1]

with W1/W2 128x128 Toeplitz matrices built on the host from the impulse
response, and windows = consecutive 128-frame chunks of the shifted input.

Precision: the harness gate is rel_err < 2e-2, so everything runs in plain
bf16 (x, W, y) with fp32 PSUM accumulation -> rel err ~2e-3 and half the
HBM traffic of an fp32/hi-lo scheme.  The whole kernel is HBM-bound:
per-core traffic is x 8.4MB + w 4.2MB + y 8.4MB = 21MB -> ~59us floor.

Window 0 of the padded input is 126 zeros + v[0:2]; instead of loading it,
its rank-2 contribution to output block 0 (together with the "base does
not exist for t<0" boundary correction) is applied on the host after
gathering.  On-chip x therefore holds exactly windows 1..8 = frames
2..1025 (1022 real + 2 zero), i.e. per (channel, window, bh):

    ps[:, 0:64]   = W2 @ win0                       (block 0, host-corrected)
    ps[:, 64:512] = W2 @ win(1..7) + W1 @ win(0..6) (blocks 1..7)

All DRAM layouts are the exact SBUF layouts (host transposes are free):
every DMA is a plain 2D column slice with 8KB contiguous runs/partition.

Sharding: channels d (512) split across 8 cores (64 each); all 64 (b,h)
sequences ride the matmul free dimension. Zero cross-device communication.

Per-core tensors:
    x  [128, 64*512] bf16   col = d*512 + win*64 + bh
    w  [128, 64*256] bf16   col = d*256 + c; c<128: W1 lhsT, c>=128: W2 lhsT
    y  [128, 64*512] bf16   col = d*512 + blk*64 + bh
"""

import numpy as np

import concourse.bass as bass
import concourse.bacc as bacc
import concourse.mybir as mybir
from concourse import tile
from concourse import bass_utils

B, H, T, D = 16, 4, 1024, 512
N_CORES = 8
DC = D // N_CORES          # 64 channels per core
BH = B * H                 # 64 sequences (matmul free dim)
NBLK = T // 128            # 8 output blocks
NWIN = 8                   # windows kept on-chip (old windows 1..8)
F32 = mybir.dt.float32
BF16 = mybir.dt.bfloat16
FREE = NBLK * BH           # 512, matmul free dim
WCOL = 192                 # w cols per channel: W2 [128x128] + W1 [64x64] block

_NC_CACHE: dict = {}


def _build_nc(dc: int = DC):
    nc = bacc.Bacc("TRN2", target_bir_lowering=False, debug=False)
    x = nc.dram_tensor("x", [128, dc * FREE], BF16, kind="ExternalInput")
    w = nc.dram_tensor("w", [128, dc * WCOL], BF16, kind="ExternalInput")
    y = nc.dram_tensor("y", [128, dc * FREE], BF16, kind="ExternalOutput")
    xap, wap, yap = x.ap(), w.ap(), y.ap()
    XG, WG, YG = 8, 32, 8              # channels per x-load / w-load / y-store

    with tile.TileContext(nc) as tc:
        with tc.tile_pool(name="xp", bufs=4) as xp, \
             tc.tile_pool(name="wp", bufs=2) as wp, \
             tc.tile_pool(name="op", bufs=3) as op, \
             tc.tile_pool(name="dp", bufs=1) as dp, \
             tc.tile_pool(name="pp", bufs=7, space="PSUM") as pp, \
             tc.tile_pool(name="pwp", bufs=1, space="PSUM") as pwp:
            # PE warm-up: the HAM clock gate keeps the PE at 1.2 GHz until it
            # sees ~3.4us of sustained activity; burn the dead DMA-head time
            # on dummy matmuls so the real stream starts at 2.4 GHz.
            dummy = dp.tile([128, FREE], BF16, name="dummy")
            nc.gpsimd.memset(dummy, 0.0)
            pw = pwp.tile([128, FREE], F32, name="pw")
            for _ in range(20):
                nc.tensor.matmul(pw, lhsT=dummy[:, 0:128], rhs=dummy,
                                 start=True, stop=True)
            # all of w (2.56MB) loads in two early DMAs and stays resident so
            # weight availability never gates the matmul stream
            whalves = []
            for g in range(dc // WG):
                wt = wp.tile([128, WG * WCOL], BF16, name="wt")
                wsrc = wap.copy()
                wsrc.ap = wsrc.ap[:0] + [[dc * WCOL, 128], [1, WG * WCOL]]
                wsrc.offset = g * WG * WCOL
                nc.scalar.dma_start(out=wt, in_=wsrc)
                whalves.append(wt)
            xt = yt = None
            for d in range(dc):
                if d % XG == 0:
                    g = d // XG
                    xt = xp.tile([128, XG * FREE], BF16, name="xt")
                    src = xap.copy()
                    src.ap = src.ap[:0] + [[dc * FREE, 128], [1, XG * FREE]]
                    src.offset = g * XG * FREE
                    nc.sync.dma_start(out=xt, in_=src)
                if d % YG == 0:
                    yt = op.tile([128, YG * FREE], BF16, name="yt")
                xv = xt[:, (d % XG) * FREE:(d % XG + 1) * FREE]
                wt = whalves[d // WG]
                wo = (d % WG) * WCOL
                w2 = wt[:, wo:wo + 128]
                w1 = wt[64:128, wo + 128:wo + 192]     # [64, 64] lhsT
                ps = pp.tile([128, FREE], F32, name="ps")
                # ps[:, 0:64] (block 0) only gets the W2 term; its group is
                # never stop'ed -> skip the sim's accumulation-group check.
                nc.tensor.matmul(ps, lhsT=w2, rhs=xv,
                                 start=True, stop=False, skip_group_check=True)
                # W1 truncated to rows [64:128] x cols [0:64] (impulse response
                # below bf16 noise past lag ~63); contraction operands both sit
                # at partitions 64:128, output covers frames 0:64 of blocks 1-7.
                nc.tensor.matmul(ps[0:BH, BH:FREE], lhsT=w1,
                                 rhs=xv[64:128, 0:FREE - BH],
                                 start=False, stop=True, skip_group_check=True)
                # alternate PSUM evacuation between DVE and ACT
                ysl = yt[:, (d % YG) * FREE:(d % YG + 1) * FREE]
                if d % 2 == 0:
                    nc.vector.tensor_copy(ysl, ps)
                else:
                    nc.scalar.copy(ysl, ps)
                if d % YG == YG - 1:
                    g = d // YG
                    dst = yap.copy()
                    dst.ap = dst.ap[:0] + [[dc * FREE, 128], [1, YG * FREE]]
                    dst.offset = g * YG * FREE
                    nc.gpsimd.dma_start(out=dst, in_=yt)
    nc.compile()
    return nc


def _get_nc(dc: int = DC):
    if dc not in _NC_CACHE:
        _NC_CACHE[dc] = _build_nc(dc)
    return _NC_CACHE[dc]


def _build_filters(l_filter: np.ndarray, r_filter: np.ndarray):
    """Returns wmat [128, 256, D] float64 (k, i; W1 = [:, :128], W2 = [:, 128:])
    and the rank-2 block-0 boundary correction corr [2, 128, D] float64."""
    c = l_filter[1:].astype(np.float64)            # (9, D) IIR coeffs
    d = c.shape[1]
    a = np.zeros((258, d))
    a[0] = 1.0
    for n in range(1, 258):
        for k in range(1, min(9, n) + 1):
            a[n] += c[k - 1] * a[n - k]
    q0 = 1.0 + l_filter[0].astype(np.float64)
    q1 = r_filter[0].astype(np.float64)
    q2 = r_filter[1].astype(np.float64)

    # wseq[lag + 129] = combined FIR tap at lag, lag in [-129, 253] (0 < -2)
    wseq = np.zeros((383, d))
    for lag in range(-2, 254):
        t = q2 * a[lag + 2]
        if lag + 1 >= 0:
            t = t + q1 * a[lag + 1]
        if lag >= 0:
            t = t + q0 * a[lag]
        wseq[lag + 129] = t

    kk = np.arange(128)[:, None]
    ii = np.arange(128)[None, :]
    w1 = wseq[ii - kk + 255]                       # (128, 128, D)
    w2 = wseq[ii - kk + 127]
    wmat = np.concatenate([w1, w2], axis=1)        # (128, 256, D)

    i1 = np.arange(128)
    corr = np.stack([-(q1[None, :] * a[i1 + 1] + q2[None, :] * a[i1 + 2]),
                     -(q2[None, :] * a[i1 + 1])], axis=0)   # (2, 128, D)
    return wmat, corr


def _make_in_maps(v, l_filter, r_filter, n_cores=N_CORES, dc=DC):
    import ml_dtypes
    bf16 = ml_dtypes.bfloat16
    wmat, _ = _build_filters(l_filter, r_filter)
    vr = np.asarray(v, dtype=np.float32).reshape(BH, T, D)
    wb = wmat.astype(np.float32).astype(bf16)      # (128, 256, D)
    # per-channel w block: [:, 0:128] = W2 lhsT (full), [64:128, 128:192] =
    # W1 lhsT truncated to rows 64:128 x cols 0:64 (rest below bf16 noise)
    warr = np.zeros((128, D, WCOL), bf16)
    warr[:, :, 0:128] = wb[:, 128:256, :].transpose(0, 2, 1)
    warr[64:128, :, 128:192] = wb[64:128, 0:64, :].transpose(0, 2, 1)

    in_maps = []
    for cid in range(n_cores):
        sl = slice(cid * dc, (cid + 1) * dc)
        vc = vr[:, :, sl].astype(bf16)             # (BH, T, dc)
        xarr = np.zeros((128, dc, NWIN, BH), bf16)
        for win in range(NWIN):
            t0 = 128 * win + 2
            n = min(128, T - t0)
            # (BH, n, dc) -> (n, dc, BH)
            xarr[:n, :, win, :] = vc[:, t0:t0 + n, :].transpose(1, 2, 0)
        in_maps.append({
            "x": np.ascontiguousarray(xarr).reshape(128, dc * FREE),
            "w": np.ascontiguousarray(warr[:, sl, :]).reshape(128, dc * WCOL),
        })
    return in_maps


def kernel(v: np.ndarray, l_filter: np.ndarray, r_filter: np.ndarray,
           **_unused) -> np.ndarray:
    nc = _get_nc(DC)
    in_maps = _make_in_maps(v, l_filter, r_filter)
    res = bass_utils.run_bass_kernel_spmd(nc, in_maps,
                                          core_ids=list(range(N_CORES)))
    vr = np.asarray(v, dtype=np.float32).reshape(BH, T, D)
    out = np.empty((BH, T, D), np.float32)
    for cid in range(N_CORES):
        yc = np.asarray(res.results[cid]["y"]).reshape(128, DC, NBLK, BH)
        # (i, d, b, j) -> (j, b, i, d) -> (BH, T, dc)
        out[:, :, cid * DC:(cid + 1) * DC] = (
            yc.astype(np.float32).transpose(3, 2, 0, 1).reshape(BH, T, DC))

    # Block-0 boundary correction: dropped window 0 (rank-2 in v[0:2]) plus
    # the "base does not exist for t<0" fix, both exact in f64 on the host.
    wmat, corr = _build_filters(l_filter, r_filter)
    cmat = wmat[126:128, 0:128, :] + corr          # (2, 128, D)
    out[:, 0:128, :] += np.einsum(
        "mid,jmd->jid", cmat, vr[:, 0:2, :].astype(np.float64)
    ).astype(np.float32)
    return out.reshape(B, H, T, D)


# revision 18
# speedup vs baseline: 1.6271x; 1.4394x over previous
"""DFSMN (order-9 IIR + 2-tap lookahead FIR along frames) on 8 Trainium2 cores.

Math: the reference computes, per (b, h, d) sequence along frames t:
    p[t] = base[t] + sum_{k=1..9} c_k[d] * p[t-k]
    base[t] = (1 + l0[d]) v[t] + r1[d] v[t+1] + r2[d] v[t+2]
This is a per-channel LTI filter, so p = w_d * v (convolution with the
filter's impulse response, which decays below bf16 resolution past lag
~120). Each 128-frame output block therefore depends only on the previous
256 input frames, which turns the whole problem into, per channel d:

    out_block(b) = W1_d^T @ x[window b] + W2_d^T @ x[window b+1]

with W1/W2 128x128 Toeplitz matrices built on the host from the impulse
response, and windows = consecutive 128-frame chunks of the shifted input.

Precision: the harness gate is rel_err < 2e-2, so everything runs in plain
bf16 (x, W, y) with fp32 PSUM accumulation -> rel err ~2e-3 and half the
HBM traffic of an fp32/hi-lo scheme.  The whole kernel is HBM-bound:
per-core traffic is x 8.4MB + w 4.2MB + y 8.4MB = 21MB -> ~59us floor.

Window 0 of the padded input is 126 zeros + v[0:2]; instead of loading it,
its rank-2 contribution to output block 0 (together with the "base does
not exist for t<0" boundary correction) is applied on the host after
gathering.  On-chip x therefore holds exactly windows 1..8 = frames
2..1025 (1022 real + 2 zero), i.e. per (channel, window, bh):

    ps[:, 0:64]   = W2 @ win0                       (block 0, host-corrected)
    ps[:, 64:512] = W2 @ win(1..7) + W1 @ win(0..6) (blocks 1..7)

All DRAM layouts are the exact SBUF layouts (host transposes are free):
every DMA is a plain 2D column slice with 8KB contiguous runs/partition.

Sharding: channels d (512) split across 8 cores (64 each); all 64 (b,h)
sequences ride the matmul free dimension. Zero cross-device communication.

Per-core tensors:
    x  [128, 64*512] bf16   col = d*512 + win*64 + bh
    w  [128, 64*256] bf16   col = d*256 + c; c<128: W1 lhsT, c>=128: W2 lhsT
    y  [128, 64*512] bf16   col = d*512 + blk*64 + bh
"""

import numpy as np

import concourse.bass as bass
import concourse.bacc as bacc
import concourse.mybir as mybir
from concourse import tile
from concourse import bass_utils

B, H, T, D = 16, 4, 1024, 512
N_CORES = 8
DC = D // N_CORES          # 64 channels per core
BH = B * H                 # 64 sequences (matmul free dim)
NBLK = T // 128            # 8 output blocks
NWIN = 8                   # windows kept on-chip (old windows 1..8)
F32 = mybir.dt.float32
BF16 = mybir.dt.bfloat16
FREE = NBLK * BH           # 512, matmul free dim
WCOL = 192                 # w cols per channel: W2 [128x128] + W1 [64x64] block

_NC_CACHE: dict = {}


def _build_nc(dc: int = DC):
    nc = bacc.Bacc("TRN2", target_bir_lowering=False, debug=False)
    x = nc.dram_tensor("x", [128, dc * FREE], BF16, kind="ExternalInput")
    w = nc.dram_tensor("w", [128, dc * WCOL], BF16, kind="ExternalInput")
    y = nc.dram_tensor("y", [128, dc * FREE], BF16, kind="ExternalOutput")
    xap, wap, yap = x.ap(), w.ap(), y.ap()
    XG, WG, YG = 8, 8, 8               # channels per x-load / w-load / y-store

    with tile.TileContext(nc) as tc:
        with tc.tile_pool(name="xp", bufs=4) as xp, \
             tc.tile_pool(name="wp", bufs=8) as wp, \
             tc.tile_pool(name="op", bufs=3) as op, \
             tc.tile_pool(name="dp", bufs=1) as dp, \
             tc.tile_pool(name="pp", bufs=7, space="PSUM") as pp, \
             tc.tile_pool(name="pwp", bufs=1, space="PSUM") as pwp:
            # PE warm-up: the HAM clock gate keeps the PE at 1.2 GHz until it
            # sees ~3.4us of sustained activity; burn the dead DMA-head time
            # on dummy matmuls so the real stream starts at 2.4 GHz.
            dummy = dp.tile([128, FREE], BF16, name="dummy")
            nc.gpsimd.memset(dummy, 0.0)
            pw = pwp.tile([128, FREE], F32, name="pw")
            for _ in range(20):
                nc.tensor.matmul(pw, lhsT=dummy[:, 0:128], rhs=dummy,
                                 start=True, stop=True)
            # all of w (2.56MB) loads in early 0.4MB DMAs and stays resident;
            # small first tile = ready by ~11us so it never gates the matmuls
            whalves = []
            for g in range(dc // WG):
                wt = wp.tile([128, WG * WCOL], BF16, name="wt")
                wsrc = wap.copy()
                wsrc.ap = wsrc.ap[:0] + [[dc * WCOL, 128], [1, WG * WCOL]]
                wsrc.offset = g * WG * WCOL
                nc.scalar.dma_start(out=wt, in_=wsrc)
                whalves.append(wt)
            xt = yt = None
            for d in range(dc):
                if d % XG == 0:
                    g = d // XG
                    xt = xp.tile([128, XG * FREE], BF16, name="xt")
                    src = xap.copy()
                    src.ap = src.ap[:0] + [[dc * FREE, 128], [1, XG * FREE]]
                    src.offset = g * XG * FREE
                    nc.sync.dma_start(out=xt, in_=src)
                if d % YG == 0:
                    yt = op.tile([128, YG * FREE], BF16, name="yt")
                xv = xt[:, (d % XG) * FREE:(d % XG + 1) * FREE]
                wt = whalves[d // WG]
                wo = (d % WG) * WCOL
                w2 = wt[:, wo:wo + 128]
                w1 = wt[64:128, wo + 128:wo + 192]     # [64, 64] lhsT
                ps = pp.tile([128, FREE], F32, name="ps")
                # ps[:, 0:64] (block 0) only gets the W2 term; its group is
                # never stop'ed -> skip the sim's accumulation-group check.
                nc.tensor.matmul(ps, lhsT=w2, rhs=xv,
                                 start=True, stop=False, skip_group_check=True)
                # W1 truncated to rows [64:128] x cols [0:64] (impulse response
                # below bf16 noise past lag ~63); contraction operands both sit
                # at partitions 64:128, output covers frames 0:64 of blocks 1-7.
                nc.tensor.matmul(ps[0:BH, BH:FREE], lhsT=w1,
                                 rhs=xv[64:128, 0:FREE - BH],
                                 start=False, stop=True, skip_group_check=True)
                # alternate PSUM evacuation between DVE and ACT
                ysl = yt[:, (d % YG) * FREE:(d % YG + 1) * FREE]
                if d % 2 == 0:
                    nc.vector.tensor_copy(ysl, ps)
                else:
                    nc.scalar.copy(ysl, ps)
                if d % YG == YG - 1:
                    g = d // YG
                    dst = yap.copy()
                    dst.ap = dst.ap[:0] + [[dc * FREE, 128], [1, YG * FREE]]
                    dst.offset = g * YG * FREE
                    nc.gpsimd.dma_start(out=dst, in_=yt)
    nc.compile()
    return nc


def _get_nc(dc: int = DC):
    if dc not in _NC_CACHE:
        _NC_CACHE[dc] = _build_nc(dc)
    return _NC_CACHE[dc]


def _build_filters(l_filter: np.ndarray, r_filter: np.ndarray):
    """Returns wmat [128, 256, D] float64 (k, i; W1 = [:, :128], W2 = [:, 128:])
    and the rank-2 block-0 boundary correction corr [2, 128, D] float64."""
    c = l_filter[1:].astype(np.float64)            # (9, D) IIR coeffs
    d = c.shape[1]
    a = np.zeros((258, d))
    a[0] = 1.0
    for n in range(1, 258):
        for k in range(1, min(9, n) + 1):
            a[n] += c[k - 1] * a[n - k]
    q0 = 1.0 + l_filter[0].astype(np.float64)
    q1 = r_filter[0].astype(np.float64)
    q2 = r_filter[1].astype(np.float64)

    # wseq[lag + 129] = combined FIR tap at lag, lag in [-129, 253] (0 < -2)
    wseq = np.zeros((383, d))
    for lag in range(-2, 254):
        t = q2 * a[lag + 2]
        if lag + 1 >= 0:
            t = t + q1 * a[lag + 1]
        if lag >= 0:
            t = t + q0 * a[lag]
        wseq[lag + 129] = t

    kk = np.arange(128)[:, None]
    ii = np.arange(128)[None, :]
    w1 = wseq[ii - kk + 255]                       # (128, 128, D)
    w2 = wseq[ii - kk + 127]
    wmat = np.concatenate([w1, w2], axis=1)        # (128, 256, D)

    i1 = np.arange(128)
    corr = np.stack([-(q1[None, :] * a[i1 + 1] + q2[None, :] * a[i1 + 2]),
                     -(q2[None, :] * a[i1 + 1])], axis=0)   # (2, 128, D)
    return wmat, corr


def _make_in_maps(v, l_filter, r_filter, n_cores=N_CORES, dc=DC):
    import ml_dtypes
    bf16 = ml_dtypes.bfloat16
    wmat, _ = _build_filters(l_filter, r_filter)
    vr = np.asarray(v, dtype=np.float32).reshape(BH, T, D)
    wb = wmat.astype(np.float32).astype(bf16)      # (128, 256, D)
    # per-channel w block: [:, 0:128] = W2 lhsT (full), [64:128, 128:192] =
    # W1 lhsT truncated to rows 64:128 x cols 0:64 (rest below bf16 noise)
    warr = np.zeros((128, D, WCOL), bf16)
    warr[:, :, 0:128] = wb[:, 128:256, :].transpose(0, 2, 1)
    warr[64:128, :, 128:192] = wb[64:128, 0:64, :].transpose(0, 2, 1)

    in_maps = []
    for cid in range(n_cores):
        sl = slice(cid * dc, (cid + 1) * dc)
        vc = vr[:, :, sl].astype(bf16)             # (BH, T, dc)
        xarr = np.zeros((128, dc, NWIN, BH), bf16)
        for win in range(NWIN):
            t0 = 128 * win + 2
            n = min(128, T - t0)
            # (BH, n, dc) -> (n, dc, BH)
            xarr[:n, :, win, :] = vc[:, t0:t0 + n, :].transpose(1, 2, 0)
        in_maps.append({
            "x": np.ascontiguousarray(xarr).reshape(128, dc * FREE),
            "w": np.ascontiguousarray(warr[:, sl, :]).reshape(128, dc * WCOL),
        })
    return in_maps


def kernel(v: np.ndarray, l_filter: np.ndarray, r_filter: np.ndarray,
           **_unused) -> np.ndarray:
    nc = _get_nc(DC)
    in_maps = _make_in_maps(v, l_filter, r_filter)
    res = bass_utils.run_bass_kernel_spmd(nc, in_maps,
                                          core_ids=list(range(N_CORES)))
    vr = np.asarray(v, dtype=np.float32).reshape(BH, T, D)
    out = np.empty((BH, T, D), np.float32)
    for cid in range(N_CORES):
        yc = np.asarray(res.results[cid]["y"]).reshape(128, DC, NBLK, BH)
        # (i, d, b, j) -> (j, b, i, d) -> (BH, T, dc)
        out[:, :, cid * DC:(cid + 1) * DC] = (
            yc.astype(np.float32).transpose(3, 2, 0, 1).reshape(BH, T, DC))

    # Block-0 boundary correction: dropped window 0 (rank-2 in v[0:2]) plus
    # the "base does not exist for t<0" fix, both exact in f64 on the host.
    wmat, corr = _build_filters(l_filter, r_filter)
    cmat = wmat[126:128, 0:128, :] + corr          # (2, 128, D)
    out[:, 0:128, :] += np.einsum(
        "mid,jmd->jid", cmat, vr[:, 0:2, :].astype(np.float64)
    ).astype(np.float32)
    return out.reshape(B, H, T, D)
